# revision 54
# baseline (speedup 1.0000x reference)
"""Multi-head attention (B=2, S=2048, E=1024, H=16) on 8 Trainium2 NeuronCores.

Sharding: data-parallel over the 2 batches x tensor-parallel over 4 head-groups
(4 heads each).  Core c handles batch c//4, heads [4*(c%4), 4*(c%4)+4).
Each core computes its heads' Q/K/V projections, softmax(QK^T/8)V, and the
partial output projection against the matching Wo column slice; the host sums
the 4 partials per batch (the head-parallel all-reduce) and stacks batches.

Device-side layout notes:
 - Host pre-transposes x and the weight slices so every matmul operand already
   has its contraction dim on SBUF partitions (no on-device transposes).
 - The whole PE stream is 16-bit: x/W/q/k/v/Wo/concat are bf16 and the exp
   output is fp16 (the activation engine writes fp16 at full rate, bf16 with a
   +20% penalty; 16-bit operands also halve LDWEIGHTS time and let the PV
   matmuls issue back-to-back at 1 column/cycle).  PSUM accumulation is fp32
   throughout, and y partials are bf16 summed in f32 on the host.
 - Scores are produced transposed, sT[j, i] = k_j . q_i, so softmax(j) is a
   partition-dim reduction folded into the P@V matmul via a ones column on V
   (out row 64 = sum_j exp(sT[j, i])), and the attention output lands directly
   in the [head_dim, seq] layout the output projection needs as lhsT.
 - exp runs on the scalar engine straight out of PSUM with the 1/sqrt(dk)
   scale and a constant -4 bias folded in (softmax is shift-invariant).
 - The two heads of a pair accumulate into separate single-bank PSUM tiles so
   the next pair's first PV only waits on the first bank's drain (the tile
   framework serializes cross-engine accesses per tile).
 - Non-attention work (projections, output tiles, softmax normalize) is cut
   into small units scheduled at explicit iterations of the attention loop,
   popped between each j-tile pair's exps and the previous pair's PVs so the
   PE column stream never goes idle (idle gaps re-throttle the HAM clock gate
   to half rate); j tiles are processed two at a time to halve the score<->PV
   weight-reload transitions, and junk warm matmuls pad the prologue
   (input-DMA wait) and the epilogue so the clock stays at full rate there.
"""

import heapq

import numpy as np
import ml_dtypes

import concourse.bass as bass
from concourse import bacc
import concourse.mybir as mybir
import concourse.tile as tile
from concourse.bass_utils import run_bass_kernel_spmd

B, S, E, H = 2, 2048, 1024, 16
DK = 64
NCORES = 8
HGROUPS = 4            # head-parallel groups per batch
HLOC = H // HGROUPS    # heads per core = 4
FH = HLOC * DK         # local feature cols = 256

F32 = mybir.dt.float32
F32R = mybir.dt.float32r
BF16 = mybir.dt.bfloat16
EXP_BIAS = -4.0        # constant shift inside exp; cancels in softmax


def _round_f32r(a: np.ndarray) -> np.ndarray:
    """Round fp32 to the bf16-pair (hi+lo) values the PE's fp32r mode uses."""
    hi = a.astype(ml_dtypes.bfloat16).astype(np.float32)
    lo = (a - hi).astype(ml_dtypes.bfloat16).astype(np.float32)
    return hi + lo


def _build_program() -> bass.Bass:
    nc = bacc.Bacc("TRN2", target_bir_lowering=False, debug=False,
                   enable_asserts=False)

    # inputs are pre-arranged on the host into the exact SBUF tile layout so
    # every input DMA is fully contiguous (8KB per partition line, vs 1KB
    # strided lines when rearranging [E, S] on the fly)
    xt_d = nc.dram_tensor("xt", [4, 128, E // 128, 512], BF16,
                          kind="ExternalInput").ap()
    wqt_d = nc.dram_tensor("wqt", [FH // 128, 128, E // 128, 128], BF16,
                           kind="ExternalInput").ap()
    wkt_d = nc.dram_tensor("wkt", [FH // 128, 128, E // 128, 128], BF16,
                           kind="ExternalInput").ap()
    wvt_d = nc.dram_tensor("wvt", [128, E // 128, FH], BF16,
                           kind="ExternalInput").ap()
    wot_d = nc.dram_tensor("wot", [128, FH // 128, E], BF16,
                           kind="ExternalInput").ap()
    y_d = nc.dram_tensor("y", [S, E], BF16, kind="ExternalOutput").ap()

    EC = E // 128        # 8 contraction chunks for the projections
    ST = S // 128        # 16 seq tiles of 128 (the j tiles)
    SC = S // 512        # 4 seq chunks of 512 (the i chunks)
    FT = FH // 128       # 2 feature tiles (head pairs)
    ITERS = SC * FT * ST   # 128 attention inner iterations

    with tile.TileContext(nc) as tc:
        with (
            tc.tile_pool(name="constp", bufs=1) as constp,
            tc.tile_pool(name="xtp", bufs=SC) as xtp,
            tc.tile_pool(name="wp", bufs=1) as wp,
            tc.tile_pool(name="qkp", bufs=2 * FT * SC) as qkp,
            tc.tile_pool(name="vp", bufs=ST) as vp,
            tc.tile_pool(name="cp", bufs=3) as cp,
            tc.tile_pool(name="ep", bufs=6) as ep,
            tc.tile_pool(name="aup", bufs=8) as aup,
            tc.tile_pool(name="smp", bufs=10) as smp,
            tc.tile_pool(name="op", bufs=6) as op,
            tc.tile_pool(name="mmp", bufs=2, space="PSUM") as mmp,
            tc.tile_pool(name="scp", bufs=2, space="PSUM") as scp,
            tc.tile_pool(name="atp", bufs=2, space="PSUM") as atp,
        ):
            ones = constp.tile([128, DK], BF16, tag="ones")
            nc.vector.memset(ones[:], 1.0)
            bias_t = constp.tile([128, 1], F32, tag="bias")
            nc.vector.memset(bias_t[:], EXP_BIAS)
            onescol = constp.tile([128, HLOC], BF16, tag="onescol")
            nc.vector.memset(onescol[:], 1.0)

            # ---- input DMAs ----
            # x arrives as 32 per-(e-chunk, s-chunk) tiles, s-chunk major, so
            # the first k/q projection groups can start a few us in instead of
            # waiting for the whole transfer (per-tile dependencies).
            # K/Q weights arrive in per-head-pair halves: the first scores
            # only need the ft0 halves, so only 1.5MB gates the first exp
            WQs = [wp.tile([128, EC, 128], BF16, tag=f"wq{ft}", name=f"wq{ft}")
                   for ft in range(FT)]
            WKs = [wp.tile([128, EC, 128], BF16, tag=f"wk{ft}", name=f"wk{ft}")
                   for ft in range(FT)]
            WV = wp.tile([128, EC, FH], BF16, tag="wv")
            WO = wp.tile([128, FT, E], BF16, tag="wo")
            nc.sync.dma_start(WKs[0][:], wkt_d[0])
            nc.sync.dma_start(WQs[0][:], wqt_d[0])
            XSC = []
            for sc in range(SC):
                xtile = xtp.tile([128, EC, 512], BF16, tag="xt",
                                 name=f"xt_{sc}")
                nc.sync.dma_start(xtile[:], xt_d[sc])
                XSC.append(xtile)
                if sc == 0:
                    nc.sync.dma_start(WV[:], wvt_d)
                    nc.sync.dma_start(WKs[1][:], wkt_d[1])
                    nc.sync.dma_start(WQs[1][:], wqt_d[1])
            # Wo is first used ~40 iterations in; keep it behind all of x.
            nc.sync.dma_start(WO[:], wot_d)

            # ---- PE warmup during the input-DMA window ----
            # The HAM clock gate starts at half clock and needs ~3.4us of
            # sustained PE activity; burn cheap bf16 matmuls on junk data
            # while the inputs stream in so the projections run warm.
            warm = constp.tile([128, 512], BF16, tag="warm")
            nc.vector.memset(warm[:], 1.0)
            ps_w = mmp.tile([128, 512], F32, tag="mm", name="warmps")
            for _ in range(20):
                nc.tensor.matmul(ps_w[:, :], warm[:, 0:128], warm[:, :],
                                 start=True, stop=True)

            # ---- filler unit scheduler ----
            # Each unit is a small PE burst; run_at is the attention
            # iteration (ic*32 + ft*16 + jt) it should run right before.
            # run_at also fixes instruction emission order, so it must
            # respect dataflow (e.g. output tiles after their normalize).
            pending = []
            seq = [0]

            def add(run_at, fn):
                heapq.heappush(pending, (run_at, seq[0], fn))
                seq[0] += 1

            def pop_units(it):
                while pending and pending[0][0] <= it:
                    heapq.heappop(pending)[2]()

            # ---- projections ----
            # qT/kT: [f, s] layout.  out[f_tile, s_chunk] = sum_ec WqT^T @ xT
            QTs = {}
            KTs = {}

            def qk_proj_one(store, w, ft, sc):
                ps = mmp.tile([128, 512], F32, tag="mm", name="qkps")
                for ec in range(EC):
                    nc.tensor.matmul(
                        ps[:, :],
                        w[ft][:, ec, :],
                        XSC[sc][:, ec, :],
                        start=(ec == 0), stop=(ec == EC - 1),
                    )
                dst = qkp.tile([128, 512], BF16, tag="qk",
                               name=f"qk_{ft}_{sc}_{len(store)}")
                nc.vector.tensor_copy(dst[:], ps[:, :])
                store[(ft, sc)] = dst

            # v: natural [s, f] layout, plus a fused ones column per head:
            # VAUG[jt] is [128, HLOC, DK+1] with [:, h, DK] == 1.
            VAUG = [None] * ST

            def v_proj(jt):
                va = vp.tile([128, HLOC, DK + 1], BF16, tag="vaug")
                nc.vector.tensor_copy(va[:, :, DK:DK + 1],
                                      onescol[:, :, None])
                ps = mmp.tile([128, 512], F32, tag="mm", name="vps")
                for ec in range(EC):
                    nc.tensor.matmul(
                        ps[:, 0:FH],
                        XSC[jt // 4][:, ec, (jt % 4) * 128:
                                     (jt % 4 + 1) * 128],
                        WV[:, ec, :],
                        start=(ec == 0), stop=(ec == EC - 1),
                    )
                nc.vector.tensor_copy(
                    va[:, :, 0:DK],
                    ps[:, 0:FH].rearrange("p (h d) -> p h d", d=DK))
                VAUG[jt] = va

            # Front-of-kernel projection schedule.  Prologue (before the
            # attention loop): KTs[(0,0)], QTs[(0,0)], VAUG[0..3].
            qk_proj_one(KTs, WKs, 0, 0)
            qk_proj_one(QTs, WQs, 0, 0)
            # VAUG[0..3] gate only the PVs, which trail by an iteration:
            # schedule them as units so the first scores/exp start sooner.
            for _jt in range(4):
                add(-1, lambda jt=_jt: v_proj(jt))

            for _sc in range(1, SC):             # KTs[(0,sc)] used from 4*sc
                add(4 * _sc - 2, lambda sc=_sc: qk_proj_one(KTs, WKs, 0, sc))
            for _jt in range(4, ST):             # VAUG[jt] used from iter jt
                add(_jt - 2, lambda jt=_jt: v_proj(jt))
            add(11, lambda: qk_proj_one(QTs, WQs, 1, 0))   # used from iter 16
            for _sc in range(SC):                # KTs[(1,sc)] used from 16+4sc
                add(13 + 4 * _sc, lambda sc=_sc: qk_proj_one(KTs, WKs, 1, sc))
            for _ic in range(1, SC):
                # QTs[(0,ic)] is used from 32*ic but scheduled right at the
                # previous chunk's second head-pair boundary: its 8 matmuls
                # are pure-PE filler while that pair's accumulator drains.
                add(32 * _ic - 16, lambda ic=_ic: qk_proj_one(QTs, WQs, 0, ic))
            for _ic in range(1, SC):
                # QTs[(1,ic)] is used from 32*ic+16 but scheduled right at
                # the 32*ic chunk boundary: its 8 matmuls keep the PE busy
                # while the previous chunk's accumulator drains.
                add(32 * _ic, lambda ic=_ic: qk_proj_one(QTs, WQs, 1, ic))

            def phase_c(ic, concat, start_at):
                # output projection: 8 tiles of [128 s, 512 e] per i chunk,
                # spread two iterations apart through the next chunk's loop.
                k = 0
                for stl in range(4):
                    st = ic * 4 + stl
                    for oc in range(2):
                        def emit(st=st, oc=oc, stl=stl, concat=concat):
                            ps_o = mmp.tile([128, 512], F32, tag="mm",
                                            name="ops")
                            for fc in range(FT):
                                nc.tensor.matmul(
                                    ps_o[:, :],
                                    concat[:, fc, stl * 128:(stl + 1) * 128],
                                    WO[:, fc, oc * 512:(oc + 1) * 512],
                                    start=(fc == 0), stop=(fc == FT - 1),
                                )
                            ot = op.tile([128, 512], BF16, tag="out")
                            if ic == SC - 1 and (stl + oc) % 2 == 0:
                                # tail tiles: the scalar engine is idle after
                                # the last exp, use it for half the drains
                                nc.scalar.activation(
                                    ot[:], ps_o[:, :],
                                    mybir.ActivationFunctionType.Copy)
                            else:
                                nc.vector.tensor_copy(ot[:], ps_o[:, :])
                            nc.sync.dma_start(
                                y_d[st * 128:(st + 1) * 128,
                                    oc * 512:(oc + 1) * 512],
                                ot[:])
                        # offsets 11/12 put the last two tiles right at the
                        # next chunk's head-pair boundary, where the PE needs
                        # filler while the accumulator drains.
                        add(start_at + (2 * k if k < 6 else k + 5), emit)
                        k += 1

            def normalize_half(concat, aus, dnh, ft, start_at):
                # fast approx reciprocal (f32, ~18 bits) of this head pair's
                # denominators (rows 0/32 of dnh), cast to bf16 for the
                # broadcast matmuls, then two broadcast-matmul + multiply
                # pairs writing this pair's concat half.
                rdf = smp.tile([33, 512], F32, tag="rdf")
                rd = smp.tile([33, 512], BF16, tag="rd")

                def recip(rdf=rdf, rd=rd, dnh=dnh):
                    nc.vector.reciprocal_approx_fast(rdf[:], dnh[:])
                    nc.vector.tensor_copy(rd[:], rdf[:])
                add(start_at, recip)

                for hs in range(2):
                    def norm_h(hs=hs, rd=rd, concat=concat, ft=ft,
                               aus=tuple(aus)):
                        pb = hs * DK
                        ps_b = mmp.tile([DK, 512], F32, tag="mm", name="bc")
                        nc.tensor.matmul(ps_b[:, :],
                                         ones[hs * 32:hs * 32 + 1, :],
                                         rd[hs * 32:hs * 32 + 1, :],
                                         start=True, stop=True,
                                         tile_position=(hs * 32, 0))
                        nc.vector.tensor_tensor(
                            concat[pb:pb + DK, ft, :], aus[hs][:, :],
                            ps_b[:, :], mybir.AluOpType.mult)
                    add(start_at + 1 + hs, norm_h)

            # ---- attention + output projection, per 512-wide i chunk ----
            # Heads are processed in pairs (partition bases 0/64) so the K=64
            # score matmuls run concurrently in distinct PE row groups.  The
            # attention PSUM is drained to SBUF right after the PV chain ends
            # so the next head pair can reuse the accumulator bank; filler
            # units are popped between the exp and the PV consumers so the
            # exp latency and the accumulator handoff are both hidden.
            for ic in range(SC):
                concat = cp.tile([128, FT, 512], BF16, tag="concat")
                base = ic * 32
                for ft in range(FT):           # head pair, fully interleaved
                    # one accumulator tile per head keeps the two PSUM banks'
                    # drain chains independent: the next pair's first PV only
                    # waits on the first bank's drain
                    ps_h = [atp.tile([128, 512], F32, tag="at",
                                     name=f"at_{ic}_{ft}_{hs}")
                            for hs in range(2)]
                    prev_pvs = []
                    for jtp in range(ST // 2):
                        # j tiles are processed two at a time: 4 score
                        # matmuls, 2 exps, the fillers, then the previous
                        # pair's 4 PV matmuls.  Batching same-type matmuls
                        # halves the score<->PV transition overhead on the
                        # PE and still hides the activation latency (each
                        # PV trails its scores by a whole pair-iteration).
                        exs = []
                        for j2 in range(2):
                            jt = 2 * jtp + j2
                            ps_s = scp.tile([128, 1024], F32, tag="sc")
                            for hs in range(2):
                                pb = hs * DK
                                nc.tensor.matmul(
                                    ps_s[:, hs * 512:(hs + 1) * 512],
                                    KTs[(ft, jt // 4)][pb:pb + DK,
                                                       (jt % 4) * 128:
                                                       (jt % 4 + 1) * 128],
                                    QTs[(ft, ic)][pb:pb + DK, :],
                                    start=True, stop=True,
                                )
                            ex = ep.tile([128, 1024], mybir.dt.float16,
                                         tag="exp")
                            nc.scalar.activation(
                                ex[:], ps_s[:],
                                mybir.ActivationFunctionType.Exp,
                                bias=bias_t[:], scale=1.0 / np.sqrt(DK))
                            exs.append(ex)
                        it = base + ft * 16 + 2 * jtp + 1
                        pop_units(it + (2 if jtp == 0 else 0))
                        for pv in prev_pvs:
                            pv()
                        prev_pvs = []
                        for j2 in range(2):
                            def pv(jt=2 * jtp + j2, ft=ft, ps_h=ps_h,
                                   ex=exs[j2]):
                                for hs in range(2):
                                    nc.tensor.matmul(
                                        ps_h[hs][0:DK + 1, :],
                                        VAUG[jt][:, ft * 2 + hs, :],
                                        ex[:, hs * 512:(hs + 1) * 512],
                                        start=(jt == 0), stop=(jt == ST - 1),
                                    )
                            prev_pvs.append(pv)
                    for pv in prev_pvs:
                        pv()
                    # free the attention psum quickly: attn rows via DVE
                    # copies, the two tiny denominator rows via DMA (fixed
                    # ~1.5us latency, but off the DVE, so the serial chain
                    # gating the next head pair's accumulator start is just
                    # the two attn copies).  At the very end of the kernel
                    # (no exp follows) one attn copy goes through the idle
                    # scalar engine as well.
                    last = (ic == SC - 1 and ft == FT - 1)
                    aus = []
                    dnh = smp.tile([33, 512], F32, tag="dn")
                    for hs in range(2):
                        au = aup.tile([DK, 512], BF16, tag="au")
                        if hs == 0 and last:
                            nc.scalar.activation(
                                au[:], ps_h[0][0:DK, :],
                                mybir.ActivationFunctionType.Copy)
                        else:
                            nc.vector.tensor_copy(au[:], ps_h[hs][0:DK, :])
                        aus.append(au)
                        if hs == 0 and last:
                            nc.scalar.activation(
                                dnh[0:1, :], ps_h[0][DK:DK + 1, :],
                                mybir.ActivationFunctionType.Copy)
                        else:
                            nc.vector.tensor_copy(
                                dnh[hs * 32:hs * 32 + 1, :],
                                ps_h[hs][DK:DK + 1, :])
                    normalize_half(concat, aus, dnh, ft,
                                   base + ft * 16 + 16 + ft)
                phase_c(ic, concat, base + 37)

            # Tail flush: the last chunk's normalize + output tiles.  The PE
            # is mostly waiting on DVE work here, which would re-throttle the
            # HAM clock gate to half rate and double every remaining matmul;
            # interleave junk warm matmuls (into a free scores bank, so they
            # wait on nothing) to hold the clock at full rate throughout.
            wps = scp.tile([128, 512], F32, tag="sc", name="tailwarm")
            for n, (_, _, fn) in enumerate(sorted(pending)):
                for _ in range(3 if n < 5 else 1):
                    nc.tensor.matmul(wps[:, :], warm[:, 0:128], warm[:, :],
                                     start=True, stop=True)
                fn()

    nc.compile()
    return nc


_PROGRAM = None


def _get_program() -> bass.Bass:
    global _PROGRAM
    if _PROGRAM is None:
        _PROGRAM = _build_program()
    return _PROGRAM


def _prepare_in_maps(x, Wq, Wk, Wv, Wo):
    x = np.asarray(x, dtype=np.float32)
    Wq = np.asarray(Wq, dtype=np.float32)
    Wk = np.asarray(Wk, dtype=np.float32)
    Wv = np.asarray(Wv, dtype=np.float32)
    Wo = np.asarray(Wo, dtype=np.float32)
    bf = ml_dtypes.bfloat16
    in_maps = []
    for c in range(NCORES):
        b, hg = c // HGROUPS, c % HGROUPS
        rows = slice(hg * FH, (hg + 1) * FH)
        # device tile layouts: x -> [sc][p, c, s], W -> [p, c, f]
        # where the contraction index e = c*128 + p
        xt = x[b].T.reshape(E // 128, 128, S)
        xt = np.stack([xt[:, :, sc * 512:(sc + 1) * 512].transpose(1, 0, 2)
                       for sc in range(4)])
        wq = Wq[rows, :].T.reshape(E // 128, 128, FH // 128, 128)
        wq = wq.transpose(2, 1, 0, 3)
        wk = Wk[rows, :].T.reshape(E // 128, 128, FH // 128, 128)
        wk = wk.transpose(2, 1, 0, 3)
        wv = Wv[rows, :].T.reshape(E // 128, 128, FH).transpose(1, 0, 2)
        wo = Wo[:, rows].T.reshape(FH // 128, 128, E).transpose(1, 0, 2)
        in_maps.append({
            "xt": np.ascontiguousarray(xt).astype(bf),
            "wqt": np.ascontiguousarray(wq).astype(bf),
            "wkt": np.ascontiguousarray(wk).astype(bf),
            "wvt": np.ascontiguousarray(wv).astype(bf),
            "wot": np.ascontiguousarray(wo).astype(bf),
        })
    return in_maps


def run(inputs: dict, **spmd_kwargs):
    """Run on all 8 cores; returns (full output, BassKernelResults)."""
    nc = _get_program()
    in_maps = _prepare_in_maps(**inputs)
    res = run_bass_kernel_spmd(nc, in_maps, core_ids=list(range(NCORES)),
                               **spmd_kwargs)
    partials = [r["y"] for r in res.results]
    out = np.empty((B, S, E), dtype=np.float32)
    for b in range(B):
        acc = partials[b * HGROUPS].astype(np.float32, copy=True)
        for hg in range(1, HGROUPS):
            acc += partials[b * HGROUPS + hg]
        out[b] = acc
    return out, res


def kernel(**inputs) -> np.ndarray:
    out, _ = run(inputs)
    return out


# revision 55
# speedup vs baseline: 1.0047x; 1.0047x over previous
"""Multi-head attention (B=2, S=2048, E=1024, H=16) on 8 Trainium2 NeuronCores.

Sharding: data-parallel over the 2 batches x tensor-parallel over 4 head-groups
(4 heads each).  Core c handles batch c//4, heads [4*(c%4), 4*(c%4)+4).
Each core computes its heads' Q/K/V projections, softmax(QK^T/8)V, and the
partial output projection against the matching Wo column slice; the host sums
the 4 partials per batch (the head-parallel all-reduce) and stacks batches.

Device-side layout notes:
 - Host pre-transposes x and the weight slices so every matmul operand already
   has its contraction dim on SBUF partitions (no on-device transposes).
 - The whole PE stream is 16-bit: x/W/q/k/v/Wo/concat are bf16 and the exp
   output is fp16 (the activation engine writes fp16 at full rate, bf16 with a
   +20% penalty; 16-bit operands also halve LDWEIGHTS time and let the PV
   matmuls issue back-to-back at 1 column/cycle).  PSUM accumulation is fp32
   throughout, and y partials are bf16 summed in f32 on the host.
 - Scores are produced transposed, sT[j, i] = k_j . q_i, so softmax(j) is a
   partition-dim reduction folded into the P@V matmul via a ones column on V
   (out row 64 = sum_j exp(sT[j, i])), and the attention output lands directly
   in the [head_dim, seq] layout the output projection needs as lhsT.
 - exp runs on the scalar engine straight out of PSUM with the 1/sqrt(dk)
   scale and a constant -4 bias folded in (softmax is shift-invariant).
 - The two heads of a pair accumulate into separate single-bank PSUM tiles so
   the next pair's first PV only waits on the first bank's drain (the tile
   framework serializes cross-engine accesses per tile).
 - Non-attention work (projections, output tiles, softmax normalize) is cut
   into small units scheduled at explicit iterations of the attention loop,
   popped between each j-tile pair's exps and the previous pair's PVs so the
   PE column stream never goes idle (idle gaps re-throttle the HAM clock gate
   to half rate); j tiles are processed two at a time to halve the score<->PV
   weight-reload transitions, and junk warm matmuls pad the prologue
   (input-DMA wait) and the epilogue so the clock stays at full rate there.
"""

import heapq

import numpy as np
import ml_dtypes

import concourse.bass as bass
from concourse import bacc
import concourse.mybir as mybir
import concourse.tile as tile
from concourse.bass_utils import run_bass_kernel_spmd

B, S, E, H = 2, 2048, 1024, 16
DK = 64
NCORES = 8
HGROUPS = 4            # head-parallel groups per batch
HLOC = H // HGROUPS    # heads per core = 4
FH = HLOC * DK         # local feature cols = 256

F32 = mybir.dt.float32
F32R = mybir.dt.float32r
BF16 = mybir.dt.bfloat16
EXP_BIAS = -4.0        # constant shift inside exp; cancels in softmax


def _round_f32r(a: np.ndarray) -> np.ndarray:
    """Round fp32 to the bf16-pair (hi+lo) values the PE's fp32r mode uses."""
    hi = a.astype(ml_dtypes.bfloat16).astype(np.float32)
    lo = (a - hi).astype(ml_dtypes.bfloat16).astype(np.float32)
    return hi + lo


def _build_program() -> bass.Bass:
    nc = bacc.Bacc("TRN2", target_bir_lowering=False, debug=False,
                   enable_asserts=False)

    # inputs are pre-arranged on the host into the exact SBUF tile layout so
    # every input DMA is fully contiguous (8KB per partition line, vs 1KB
    # strided lines when rearranging [E, S] on the fly)
    xt_d = nc.dram_tensor("xt", [4, 128, E // 128, 512], BF16,
                          kind="ExternalInput").ap()
    wqt_d = nc.dram_tensor("wqt", [128, E // 128, FH], BF16,
                           kind="ExternalInput").ap()
    wkt_d = nc.dram_tensor("wkt", [128, E // 128, FH], BF16,
                           kind="ExternalInput").ap()
    wvt_d = nc.dram_tensor("wvt", [128, E // 128, FH], BF16,
                           kind="ExternalInput").ap()
    wot_d = nc.dram_tensor("wot", [128, FH // 128, E], BF16,
                           kind="ExternalInput").ap()
    y_d = nc.dram_tensor("y", [S, E], BF16, kind="ExternalOutput").ap()

    EC = E // 128        # 8 contraction chunks for the projections
    ST = S // 128        # 16 seq tiles of 128 (the j tiles)
    SC = S // 512        # 4 seq chunks of 512 (the i chunks)
    FT = FH // 128       # 2 feature tiles (head pairs)
    ITERS = SC * FT * ST   # 128 attention inner iterations

    with tile.TileContext(nc) as tc:
        with (
            tc.tile_pool(name="constp", bufs=1) as constp,
            tc.tile_pool(name="xtp", bufs=SC) as xtp,
            tc.tile_pool(name="wp", bufs=1) as wp,
            tc.tile_pool(name="qkp", bufs=2 * FT * SC) as qkp,
            tc.tile_pool(name="vp", bufs=ST) as vp,
            tc.tile_pool(name="cp", bufs=3) as cp,
            tc.tile_pool(name="ep", bufs=6) as ep,
            tc.tile_pool(name="aup", bufs=8) as aup,
            tc.tile_pool(name="smp", bufs=10) as smp,
            tc.tile_pool(name="op", bufs=6) as op,
            tc.tile_pool(name="mmp", bufs=2, space="PSUM") as mmp,
            tc.tile_pool(name="scp", bufs=2, space="PSUM") as scp,
            tc.tile_pool(name="atp", bufs=2, space="PSUM") as atp,
        ):
            ones = constp.tile([128, DK], BF16, tag="ones")
            nc.vector.memset(ones[:], 1.0)
            bias_t = constp.tile([128, 1], F32, tag="bias")
            nc.vector.memset(bias_t[:], EXP_BIAS)
            onescol = constp.tile([128, HLOC], BF16, tag="onescol")
            nc.vector.memset(onescol[:], 1.0)

            # ---- input DMAs ----
            # x arrives as 32 per-(e-chunk, s-chunk) tiles, s-chunk major, so
            # the first k/q projection groups can start a few us in instead of
            # waiting for the whole transfer (per-tile dependencies).
            WQ = wp.tile([128, EC, FH], BF16, tag="wq")
            WK = wp.tile([128, EC, FH], BF16, tag="wk")
            WV = wp.tile([128, EC, FH], BF16, tag="wv")
            WO = wp.tile([128, FT, E], BF16, tag="wo")
            nc.sync.dma_start(WK[:], wkt_d)
            XSC = []
            for sc in range(SC):
                xtile = xtp.tile([128, EC, 512], BF16, tag="xt",
                                 name=f"xt_{sc}")
                nc.sync.dma_start(xtile[:], xt_d[sc])
                XSC.append(xtile)
                if sc == 0:
                    nc.sync.dma_start(WQ[:], wqt_d)
                    nc.sync.dma_start(WV[:], wvt_d)
            # Wo is first used ~40 iterations in; keep it behind all of x.
            nc.sync.dma_start(WO[:], wot_d)

            # ---- PE warmup during the input-DMA window ----
            # The HAM clock gate starts at half clock and needs ~3.4us of
            # sustained PE activity; burn cheap bf16 matmuls on junk data
            # while the inputs stream in so the projections run warm.
            warm = constp.tile([128, 512], BF16, tag="warm")
            nc.vector.memset(warm[:], 1.0)
            ps_w = mmp.tile([128, 512], F32, tag="mm", name="warmps")
            for _ in range(20):
                nc.tensor.matmul(ps_w[:, :], warm[:, 0:128], warm[:, :],
                                 start=True, stop=True)

            # ---- filler unit scheduler ----
            # Each unit is a small PE burst; run_at is the attention
            # iteration (ic*32 + ft*16 + jt) it should run right before.
            # run_at also fixes instruction emission order, so it must
            # respect dataflow (e.g. output tiles after their normalize).
            pending = []
            seq = [0]

            def add(run_at, fn):
                heapq.heappush(pending, (run_at, seq[0], fn))
                seq[0] += 1

            def pop_units(it):
                while pending and pending[0][0] <= it:
                    heapq.heappop(pending)[2]()

            # ---- projections ----
            # qT/kT: [f, s] layout.  out[f_tile, s_chunk] = sum_ec WqT^T @ xT
            QTs = {}
            KTs = {}

            def qk_proj_one(store, w, ft, sc):
                ps = mmp.tile([128, 512], F32, tag="mm", name="qkps")
                for ec in range(EC):
                    nc.tensor.matmul(
                        ps[:, :],
                        w[:, ec, ft * 128:(ft + 1) * 128],
                        XSC[sc][:, ec, :],
                        start=(ec == 0), stop=(ec == EC - 1),
                    )
                dst = qkp.tile([128, 512], BF16, tag="qk",
                               name=f"qk_{ft}_{sc}_{len(store)}")
                nc.vector.tensor_copy(dst[:], ps[:, :])
                store[(ft, sc)] = dst

            # v: natural [s, f] layout, plus a fused ones column per head:
            # VAUG[jt] is [128, HLOC, DK+1] with [:, h, DK] == 1.
            VAUG = [None] * ST

            def v_proj(jt):
                va = vp.tile([128, HLOC, DK + 1], BF16, tag="vaug")
                nc.vector.tensor_copy(va[:, :, DK:DK + 1],
                                      onescol[:, :, None])
                ps = mmp.tile([128, 512], F32, tag="mm", name="vps")
                for ec in range(EC):
                    nc.tensor.matmul(
                        ps[:, 0:FH],
                        XSC[jt // 4][:, ec, (jt % 4) * 128:
                                     (jt % 4 + 1) * 128],
                        WV[:, ec, :],
                        start=(ec == 0), stop=(ec == EC - 1),
                    )
                nc.vector.tensor_copy(
                    va[:, :, 0:DK],
                    ps[:, 0:FH].rearrange("p (h d) -> p h d", d=DK))
                VAUG[jt] = va

            # Front-of-kernel projection schedule.  Prologue (before the
            # attention loop): KTs[(0,0)], QTs[(0,0)], VAUG[0..3].
            qk_proj_one(KTs, WK, 0, 0)
            qk_proj_one(QTs, WQ, 0, 0)
            # VAUG[0..3] gate only the PVs, which trail by an iteration:
            # schedule them as units so the first scores/exp start sooner.
            for _jt in range(4):
                add(-1, lambda jt=_jt: v_proj(jt))

            for _sc in range(1, SC):             # KTs[(0,sc)] used from 4*sc
                add(4 * _sc - 2, lambda sc=_sc: qk_proj_one(KTs, WK, 0, sc))
            for _jt in range(4, ST):             # VAUG[jt] used from iter jt
                add(_jt - 2, lambda jt=_jt: v_proj(jt))
            add(11, lambda: qk_proj_one(QTs, WQ, 1, 0))   # used from iter 16
            for _sc in range(SC):                # KTs[(1,sc)] used from 16+4sc
                add(13 + 4 * _sc, lambda sc=_sc: qk_proj_one(KTs, WK, 1, sc))
            for _ic in range(1, SC):
                # QTs[(0,ic)] is used from 32*ic but scheduled right at the
                # previous chunk's second head-pair boundary: its 8 matmuls
                # are pure-PE filler while that pair's accumulator drains.
                add(32 * _ic - 16, lambda ic=_ic: qk_proj_one(QTs, WQ, 0, ic))
            for _ic in range(1, SC):
                # QTs[(1,ic)] is used from 32*ic+16 but scheduled right at
                # the 32*ic chunk boundary: its 8 matmuls keep the PE busy
                # while the previous chunk's accumulator drains.
                add(32 * _ic, lambda ic=_ic: qk_proj_one(QTs, WQ, 1, ic))

            def phase_c(ic, concat, start_at):
                # output projection: 8 tiles of [128 s, 512 e] per i chunk,
                # spread two iterations apart through the next chunk's loop.
                k = 0
                for stl in range(4):
                    st = ic * 4 + stl
                    for oc in range(2):
                        def emit(st=st, oc=oc, stl=stl, concat=concat):
                            ps_o = mmp.tile([128, 512], F32, tag="mm",
                                            name="ops")
                            for fc in range(FT):
                                nc.tensor.matmul(
                                    ps_o[:, :],
                                    concat[:, fc, stl * 128:(stl + 1) * 128],
                                    WO[:, fc, oc * 512:(oc + 1) * 512],
                                    start=(fc == 0), stop=(fc == FT - 1),
                                )
                            ot = op.tile([128, 512], BF16, tag="out")
                            if ic == SC - 1 and (stl + oc) % 2 == 0:
                                # tail tiles: the scalar engine is idle after
                                # the last exp, use it for half the drains
                                nc.scalar.activation(
                                    ot[:], ps_o[:, :],
                                    mybir.ActivationFunctionType.Copy)
                            else:
                                nc.vector.tensor_copy(ot[:], ps_o[:, :])
                            nc.sync.dma_start(
                                y_d[st * 128:(st + 1) * 128,
                                    oc * 512:(oc + 1) * 512],
                                ot[:])
                        # offsets 11/12 put the last two tiles right at the
                        # next chunk's head-pair boundary, where the PE needs
                        # filler while the accumulator drains.
                        add(start_at + (2 * k if k < 6 else k + 5), emit)
                        k += 1

            def normalize_half(concat, aus, dnh, ft, start_at):
                # fast approx reciprocal (f32, ~18 bits) of this head pair's
                # denominators (rows 0/32 of dnh), cast to bf16 for the
                # broadcast matmuls, then two broadcast-matmul + multiply
                # pairs writing this pair's concat half.
                rdf = smp.tile([33, 512], F32, tag="rdf")
                rd = smp.tile([33, 512], BF16, tag="rd")

                def recip(rdf=rdf, rd=rd, dnh=dnh):
                    nc.vector.reciprocal_approx_fast(rdf[:], dnh[:])
                    nc.vector.tensor_copy(rd[:], rdf[:])
                add(start_at, recip)

                for hs in range(2):
                    def norm_h(hs=hs, rd=rd, concat=concat, ft=ft,
                               aus=tuple(aus)):
                        pb = hs * DK
                        ps_b = mmp.tile([DK, 512], F32, tag="mm", name="bc")
                        nc.tensor.matmul(ps_b[:, :],
                                         ones[hs * 32:hs * 32 + 1, :],
                                         rd[hs * 32:hs * 32 + 1, :],
                                         start=True, stop=True,
                                         tile_position=(hs * 32, 0))
                        nc.vector.tensor_tensor(
                            concat[pb:pb + DK, ft, :], aus[hs][:, :],
                            ps_b[:, :], mybir.AluOpType.mult)
                    add(start_at + 1 + hs, norm_h)

            # ---- attention + output projection, per 512-wide i chunk ----
            # Heads are processed in pairs (partition bases 0/64) so the K=64
            # score matmuls run concurrently in distinct PE row groups.  The
            # attention PSUM is drained to SBUF right after the PV chain ends
            # so the next head pair can reuse the accumulator bank; filler
            # units are popped between the exp and the PV consumers so the
            # exp latency and the accumulator handoff are both hidden.
            for ic in range(SC):
                concat = cp.tile([128, FT, 512], BF16, tag="concat")
                base = ic * 32
                for ft in range(FT):           # head pair, fully interleaved
                    # one accumulator tile per head keeps the two PSUM banks'
                    # drain chains independent: the next pair's first PV only
                    # waits on the first bank's drain
                    ps_h = [atp.tile([128, 512], F32, tag="at",
                                     name=f"at_{ic}_{ft}_{hs}")
                            for hs in range(2)]
                    prev_pvs = []
                    for jtp in range(ST // 2):
                        # j tiles are processed two at a time: 4 score
                        # matmuls, 2 exps, the fillers, then the previous
                        # pair's 4 PV matmuls.  Batching same-type matmuls
                        # halves the score<->PV transition overhead on the
                        # PE and still hides the activation latency (each
                        # PV trails its scores by a whole pair-iteration).
                        exs = []
                        for j2 in range(2):
                            jt = 2 * jtp + j2
                            ps_s = scp.tile([128, 1024], F32, tag="sc")
                            for hs in range(2):
                                pb = hs * DK
                                nc.tensor.matmul(
                                    ps_s[:, hs * 512:(hs + 1) * 512],
                                    KTs[(ft, jt // 4)][pb:pb + DK,
                                                       (jt % 4) * 128:
                                                       (jt % 4 + 1) * 128],
                                    QTs[(ft, ic)][pb:pb + DK, :],
                                    start=True, stop=True,
                                )
                            ex = ep.tile([128, 1024], mybir.dt.float16,
                                         tag="exp")
                            nc.scalar.activation(
                                ex[:], ps_s[:],
                                mybir.ActivationFunctionType.Exp,
                                bias=bias_t[:], scale=1.0 / np.sqrt(DK))
                            exs.append(ex)
                        it = base + ft * 16 + 2 * jtp + 1
                        pop_units(it + (2 if jtp == 0 else 0))
                        for pv in prev_pvs:
                            pv()
                        prev_pvs = []
                        for j2 in range(2):
                            def pv(jt=2 * jtp + j2, ft=ft, ps_h=ps_h,
                                   ex=exs[j2]):
                                for hs in range(2):
                                    nc.tensor.matmul(
                                        ps_h[hs][0:DK + 1, :],
                                        VAUG[jt][:, ft * 2 + hs, :],
                                        ex[:, hs * 512:(hs + 1) * 512],
                                        start=(jt == 0), stop=(jt == ST - 1),
                                    )
                            prev_pvs.append(pv)
                    for pv in prev_pvs:
                        pv()
                    # free the attention psum quickly: attn rows via DVE
                    # copies, the two tiny denominator rows via DMA (fixed
                    # ~1.5us latency, but off the DVE, so the serial chain
                    # gating the next head pair's accumulator start is just
                    # the two attn copies).  At the very end of the kernel
                    # (no exp follows) one attn copy goes through the idle
                    # scalar engine as well.
                    last = (ic == SC - 1 and ft == FT - 1)
                    aus = []
                    dnh = smp.tile([33, 512], F32, tag="dn")
                    for hs in range(2):
                        au = aup.tile([DK, 512], BF16, tag="au")
                        if hs == 0 and last:
                            nc.scalar.activation(
                                au[:], ps_h[0][0:DK, :],
                                mybir.ActivationFunctionType.Copy)
                        else:
                            nc.vector.tensor_copy(au[:], ps_h[hs][0:DK, :])
                        aus.append(au)
                        if hs == 0 and last:
                            nc.scalar.activation(
                                dnh[0:1, :], ps_h[0][DK:DK + 1, :],
                                mybir.ActivationFunctionType.Copy)
                        else:
                            nc.vector.tensor_copy(
                                dnh[hs * 32:hs * 32 + 1, :],
                                ps_h[hs][DK:DK + 1, :])
                    normalize_half(concat, aus, dnh, ft,
                                   base + ft * 16 + 16 + ft)
                phase_c(ic, concat, base + 37)

            # Tail flush: the last chunk's normalize + output tiles.  The PE
            # is mostly waiting on DVE work here, which would re-throttle the
            # HAM clock gate to half rate and double every remaining matmul;
            # interleave junk warm matmuls (into a free scores bank, so they
            # wait on nothing) to hold the clock at full rate throughout.
            wps = scp.tile([128, 512], F32, tag="sc", name="tailwarm")
            for n, (_, _, fn) in enumerate(sorted(pending)):
                for _ in range(3 if n < 5 else 1):
                    nc.tensor.matmul(wps[:, :], warm[:, 0:128], warm[:, :],
                                     start=True, stop=True)
                fn()

    nc.compile()
    return nc


_PROGRAM = None


def _get_program() -> bass.Bass:
    global _PROGRAM
    if _PROGRAM is None:
        _PROGRAM = _build_program()
    return _PROGRAM


def _prepare_in_maps(x, Wq, Wk, Wv, Wo):
    x = np.asarray(x, dtype=np.float32)
    Wq = np.asarray(Wq, dtype=np.float32)
    Wk = np.asarray(Wk, dtype=np.float32)
    Wv = np.asarray(Wv, dtype=np.float32)
    Wo = np.asarray(Wo, dtype=np.float32)
    bf = ml_dtypes.bfloat16
    in_maps = []
    for c in range(NCORES):
        b, hg = c // HGROUPS, c % HGROUPS
        rows = slice(hg * FH, (hg + 1) * FH)
        # device tile layouts: x -> [sc][p, c, s], W -> [p, c, f]
        # where the contraction index e = c*128 + p
        xt = x[b].T.reshape(E // 128, 128, S)
        xt = np.stack([xt[:, :, sc * 512:(sc + 1) * 512].transpose(1, 0, 2)
                       for sc in range(4)])
        wq = Wq[rows, :].T.reshape(E // 128, 128, FH).transpose(1, 0, 2)
        wk = Wk[rows, :].T.reshape(E // 128, 128, FH).transpose(1, 0, 2)
        wv = Wv[rows, :].T.reshape(E // 128, 128, FH).transpose(1, 0, 2)
        wo = Wo[:, rows].T.reshape(FH // 128, 128, E).transpose(1, 0, 2)
        in_maps.append({
            "xt": np.ascontiguousarray(xt).astype(bf),
            "wqt": np.ascontiguousarray(wq).astype(bf),
            "wkt": np.ascontiguousarray(wk).astype(bf),
            "wvt": np.ascontiguousarray(wv).astype(bf),
            "wot": np.ascontiguousarray(wo).astype(bf),
        })
    return in_maps


def run(inputs: dict, **spmd_kwargs):
    """Run on all 8 cores; returns (full output, BassKernelResults)."""
    nc = _get_program()
    in_maps = _prepare_in_maps(**inputs)
    res = run_bass_kernel_spmd(nc, in_maps, core_ids=list(range(NCORES)),
                               **spmd_kwargs)
    partials = [r["y"] for r in res.results]
    out = np.empty((B, S, E), dtype=np.float32)
    for b in range(B):
        acc = partials[b * HGROUPS].astype(np.float32, copy=True)
        for hg in range(1, HGROUPS):
            acc += partials[b * HGROUPS + hg]
        out[b] = acc
    return out, res


def kernel(**inputs) -> np.ndarray:
    out, _ = run(inputs)
    return out
